# revision 1
# baseline (speedup 1.0000x reference)
"""Two-layer DGL-style GCN (norm='both') on 8 TRN2 NeuronCores.

Sharding: dst-node blocks of 12544 per core (98 tiles of 128 nodes).
Per layer: project (PE matmul) -> gather per-edge src rows (indirect DMA)
-> segment-sum via one-hot matmul into PSUM -> scale by in-degree isqrt.
Layer-2 projections are exchanged with an ncfw AllGather between layers.

kernel(**inputs) takes the full unsharded inputs and returns the full
output; all sharding happens inside.
"""

import math

import numpy as np

import concourse.bacc as bacc
import concourse.bass as bass
import concourse.bass_utils as bass_utils
import concourse.mybir as mybir
import concourse.tile as tile

P = 128

# Full-problem constants (the grading harness calls kernel() with these shapes).
N_NODES = 100000
N_EDGES = 1600000
C_IN = 128
C_HID = 128
C_OUT = 40
C_OUT_PAD = 64
N_CORES = 8

F16 = mybir.dt.float16
F32 = mybir.dt.float32
I32 = mybir.dt.int32

# set by test.py to request a profiled run
TRACE = False
LAST_RESULTS = None


# ---------------------------------------------------------------- host prep


def prep_inputs(x, edge_index, W1, W2, ncores):
    """Shard the full inputs -> (in_maps, meta)."""
    n, cin = x.shape
    chid = W1.shape[1]
    cout = W2.shape[1]
    coutp = max(P // 2, int(2 ** math.ceil(math.log2(max(cout, 1)))))
    e = edge_index.shape[1]

    ntiles_pc = math.ceil(n / (ncores * P))  # tiles per core
    nb = ntiles_pc * P  # nodes per core
    npad = nb * ncores
    ntt = ntiles_pc * ncores  # total dst tiles

    src = np.asarray(edge_index[0], dtype=np.int64)
    dst = np.asarray(edge_index[1], dtype=np.int64)

    deg_out = np.bincount(src, minlength=npad).astype(np.float32)
    deg_in = np.bincount(dst, minlength=npad).astype(np.float32)
    oi = 1.0 / np.sqrt(np.maximum(deg_out, 1.0))
    ii = 1.0 / np.sqrt(np.maximum(deg_in, 1.0))

    # fold the out-degree scale into x; send transposed fp16
    xs = np.asarray(x, dtype=np.float32) * oi[:n, None]
    xsT = np.zeros((cin, npad), dtype=np.float16)
    xsT[:, :n] = xs.T

    # bucket edges by dst tile, sort by src within a tile for HBM locality
    gt = dst // P
    order = np.lexsort((src, gt))
    gts = gt[order]
    srcs = src[order].astype(np.int32)
    dls = (dst[order] % P).astype(np.float32)

    counts = np.bincount(gts, minlength=ntt)
    n_c = max(1, int(math.ceil(counts.max() / P)))
    L = n_c * P

    starts = np.zeros(ntt + 1, dtype=np.int64)
    starts[1:] = np.cumsum(counts)
    pos = np.arange(e, dtype=np.int64) - starts[gts]
    flat = gts * L + pos

    src_arr = np.zeros(ntt * L, dtype=np.int32)  # pad: gather row 0 (harmless)
    dl_arr = np.full(ntt * L, -1.0, dtype=np.float32)  # pad: matches no slot
    src_arr[flat] = srcs
    dl_arr[flat] = dls

    # [ntt, L] -> [ncores, P, ntiles_pc * n_c] with [p, t*n_c + j] = edge j*128+p
    src_pc = (
        src_arr.reshape(ncores, ntiles_pc, n_c, P)
        .transpose(0, 3, 1, 2)
        .reshape(ncores, P, ntiles_pc * n_c)
        .copy()
    )
    dl_pc = (
        dl_arr.reshape(ncores, ntiles_pc, n_c, P)
        .transpose(0, 3, 1, 2)
        .reshape(ncores, P, ntiles_pc * n_c)
        .copy()
    )

    # per-partition scale vectors: [p, t] = value for node t*128+p of the block
    iio = (ii * oi).reshape(ncores, ntiles_pc, P).transpose(0, 2, 1).copy()
    ii2 = ii.reshape(ncores, ntiles_pc, P).transpose(0, 2, 1).copy()

    W1_16 = np.asarray(W1, dtype=np.float16)
    W2p = np.zeros((chid, coutp), dtype=np.float16)
    W2p[:, :cout] = np.asarray(W2, dtype=np.float16)

    iota = np.tile(np.arange(P, dtype=np.float32), (P, 1))
    ident = np.eye(P, dtype=np.float32)

    in_maps = [
        {
            "xsT": xsT,
            "src_idx": src_pc[c],
            "dstloc": dl_pc[c],
            "iio": iio[c].astype(np.float32),
            "ii2": ii2[c].astype(np.float32),
            "W1": W1_16,
            "W2": W2p,
            "iota": iota,
            "ident": ident,
        }
        for c in range(ncores)
    ]

    meta = dict(
        n=n, cin=cin, chid=chid, cout=cout, coutp=coutp,
        ncores=ncores, ntiles_pc=ntiles_pc, nb=nb, npad=npad, n_c=n_c,
    )
    return in_maps, meta


# ---------------------------------------------------------------- device program


def build_nc(meta, debug=False, enable_asserts=False):
    cin = meta["cin"]
    chid = meta["chid"]
    coutp = meta["coutp"]
    ncores = meta["ncores"]
    ntiles_pc = meta["ntiles_pc"]
    nb = meta["nb"]
    npad = meta["npad"]
    n_c = meta["n_c"]
    ntt = npad // P

    nc = bacc.Bacc(
        "TRN2",
        target_bir_lowering=False,
        debug=debug,
        enable_asserts=enable_asserts,
        num_devices=ncores,
    )

    xsT = nc.dram_tensor("xsT", [cin, npad], F16, kind="ExternalInput")
    src_idx = nc.dram_tensor("src_idx", [P, ntiles_pc * n_c], I32, kind="ExternalInput")
    dstloc = nc.dram_tensor("dstloc", [P, ntiles_pc * n_c], F32, kind="ExternalInput")
    iio = nc.dram_tensor("iio", [P, ntiles_pc], F32, kind="ExternalInput")
    ii2 = nc.dram_tensor("ii2", [P, ntiles_pc], F32, kind="ExternalInput")
    W1 = nc.dram_tensor("W1", [cin, chid], F16, kind="ExternalInput")
    W2 = nc.dram_tensor("W2", [chid, coutp], F16, kind="ExternalInput")
    iota_d = nc.dram_tensor("iota", [P, P], F32, kind="ExternalInput")
    ident_d = nc.dram_tensor("ident", [P, P], F32, kind="ExternalInput")

    out = nc.dram_tensor("out", [nb, coutp], F32, kind="ExternalOutput")

    H1 = nc.dram_tensor("H1", [npad, chid], F16)
    H2b = nc.dram_tensor("H2b", [nb, coutp], F16)
    H2f = nc.dram_tensor("H2f", [npad, coutp], F16, addr_space="Shared")

    GRP = 4
    assert ntt % GRP == 0

    with tile.TileContext(nc) as tc:
        with (
            tc.tile_pool(name="const", bufs=1) as cpool,
            tc.tile_pool(name="a_in", bufs=3) as apool,
            tc.tile_pool(name="a_out", bufs=3) as aopool,
            tc.tile_pool(name="a_ps", bufs=2, space="PSUM") as apspool,
            tc.tile_pool(name="gbuf", bufs=2) as gpool,
            tc.tile_pool(name="mbuf", bufs=4) as mpool,
            tc.tile_pool(name="agg_ps", bufs=2, space="PSUM") as pspool,
            tc.tile_pool(name="aux_ps", bufs=1, space="PSUM") as xpspool,
            tc.tile_pool(name="flush", bufs=3) as flpool,
        ):
            w1_sb = cpool.tile([cin, chid], F16)
            nc.sync.dma_start(w1_sb[:], W1.ap())
            w2_sb = cpool.tile([chid, coutp], F16)
            nc.sync.dma_start(w2_sb[:], W2.ap())
            iota_f = cpool.tile([P, P], F32)
            nc.sync.dma_start(iota_f[:], iota_d.ap())
            ident_f = cpool.tile([P, P], F32)
            nc.sync.dma_start(ident_f[:], ident_d.ap())
            idx_all = cpool.tile([P, ntiles_pc * n_c], I32)
            nc.sync.dma_start(idx_all[:], src_idx.ap())
            dl_all = cpool.tile([P, ntiles_pc * n_c], F32)
            nc.sync.dma_start(dl_all[:], dstloc.ap())
            iio_sb = cpool.tile([P, ntiles_pc], F32)
            nc.sync.dma_start(iio_sb[:], iio.ap())
            ii2_sb = cpool.tile([P, ntiles_pc], F32)
            nc.sync.dma_start(ii2_sb[:], ii2.ap())

            # ---- Phase A: H1 = xs @ W1 for all npad rows (replicated work)
            xsTv = xsT.ap().rearrange("c (g n) -> g c n", n=GRP * P)
            h1v = H1.ap().rearrange("(g i p) c -> g p i c", p=P, i=GRP)
            for g in range(ntt // GRP):
                xt = apool.tile([cin, GRP * P], F16)
                nc.sync.dma_start(xt[:], xsTv[g])
                hp = apspool.tile([P, GRP * chid], F32)
                for i in range(GRP):
                    nc.tensor.matmul(
                        hp[:, i * chid : (i + 1) * chid],
                        lhsT=xt[:, i * P : (i + 1) * P],
                        rhs=w1_sb[:],
                        start=True,
                        stop=True,
                    )
                hs = aopool.tile([P, GRP * chid], F16)
                nc.vector.tensor_copy(hs[:], hp[:])
                nc.sync.dma_start(
                    h1v[g], hs[:].rearrange("p (i c) -> p i c", i=GRP)
                )

            # ---- Phase B: layer-1 aggregate per dst tile + fused layer-2 project
            h1_ap = H1.ap()
            for t in range(ntiles_pc):
                G = gpool.tile([P, n_c * chid], F16, tag="g1")
                # HW indirect DMA honors ONE offset per dest partition, so
                # gather chunk-by-chunk (128 rows per instruction).
                for c in range(n_c):
                    nc.gpsimd.indirect_dma_start(
                        out=G[:, c * chid : (c + 1) * chid],
                        out_offset=None,
                        in_=h1_ap,
                        in_offset=bass.IndirectOffsetOnAxis(
                            ap=idx_all[:, t * n_c + c : t * n_c + c + 1], axis=0
                        ),
                    )
                ps = pspool.tile([P, chid], F32, tag="agg1")
                for c in range(n_c):
                    M = mpool.tile([P, P], F16)
                    nc.vector.tensor_scalar(
                        out=M[:],
                        in0=iota_f[:],
                        scalar1=dl_all[:, t * n_c + c : t * n_c + c + 1],
                        scalar2=None,
                        op0=mybir.AluOpType.is_equal,
                    )
                    nc.tensor.matmul(
                        ps[:],
                        lhsT=M[:],
                        rhs=G[:, c * chid : (c + 1) * chid],
                        start=(c == 0),
                        stop=(c == n_c - 1),
                    )
                # x2s = relu(agg * ii) * oi  (positive scales commute with relu)
                x2s = flpool.tile([P, chid], F32, tag="x2s")
                nc.scalar.activation(
                    x2s[:],
                    ps[:],
                    mybir.ActivationFunctionType.Relu,
                    scale=iio_sb[:, t : t + 1],
                )
                xtp = xpspool.tile([P, P], F32, tag="xt_ps")
                nc.tensor.transpose(xtp[:], x2s[:], ident_f[:])
                xts = flpool.tile([P, P], F16, tag="xts")
                nc.vector.tensor_copy(xts[:], xtp[:])
                h2p = xpspool.tile([P, coutp], F32, tag="h2_ps")
                nc.tensor.matmul(h2p[:], lhsT=xts[:], rhs=w2_sb[:], start=True, stop=True)
                h2s = flpool.tile([P, coutp], F16, tag="h2s")
                nc.vector.tensor_copy(h2s[:], h2p[:])
                nc.sync.dma_start(H2b.ap()[t * P : (t + 1) * P, :], h2s[:])

            # ---- Phase C: exchange layer-2 projections
            nc.gpsimd.collective_compute(
                "AllGather",
                mybir.AluOpType.bypass,
                replica_groups=[list(range(ncores))],
                ins=[H2b.ap().opt()],
                outs=[H2f.ap().opt()],
            )

            # ---- Phase D: layer-2 aggregate per dst tile
            h2f_ap = H2f.ap()
            for t in range(ntiles_pc):
                G2 = gpool.tile([P, n_c * coutp], F16, tag="g2")
                for c in range(n_c):
                    nc.gpsimd.indirect_dma_start(
                        out=G2[:, c * coutp : (c + 1) * coutp],
                        out_offset=None,
                        in_=h2f_ap,
                        in_offset=bass.IndirectOffsetOnAxis(
                            ap=idx_all[:, t * n_c + c : t * n_c + c + 1], axis=0
                        ),
                    )
                ps2 = pspool.tile([P, coutp], F32, tag="agg2")
                for c in range(n_c):
                    M = mpool.tile([P, P], F16)
                    nc.vector.tensor_scalar(
                        out=M[:],
                        in0=iota_f[:],
                        scalar1=dl_all[:, t * n_c + c : t * n_c + c + 1],
                        scalar2=None,
                        op0=mybir.AluOpType.is_equal,
                    )
                    nc.tensor.matmul(
                        ps2[:],
                        lhsT=M[:],
                        rhs=G2[:, c * coutp : (c + 1) * coutp],
                        start=(c == 0),
                        stop=(c == n_c - 1),
                    )
                osb = flpool.tile([P, coutp], F32, tag="osb")
                nc.vector.tensor_scalar(
                    out=osb[:],
                    in0=ps2[:],
                    scalar1=ii2_sb[:, t : t + 1],
                    scalar2=None,
                    op0=mybir.AluOpType.mult,
                )
                nc.sync.dma_start(out.ap()[t * P : (t + 1) * P, :], osb[:])

    nc.compile()
    return nc


# ---------------------------------------------------------------- entry point

_CACHE = {}


def kernel(x, edge_index, W1, W2):
    global LAST_RESULTS
    x = np.asarray(x)
    edge_index = np.asarray(edge_index)
    W1 = np.asarray(W1)
    W2 = np.asarray(W2)

    in_maps, meta = prep_inputs(x, edge_index, W1, W2, N_CORES)

    key = (meta["npad"], meta["n_c"], meta["coutp"])
    nc = _CACHE.get(key)
    if nc is None:
        nc = build_nc(meta, debug=False, enable_asserts=False)
        _CACHE[key] = nc

    res = bass_utils.run_bass_kernel_spmd(
        nc,
        in_maps,
        core_ids=list(range(meta["ncores"])),
        trace=TRACE,
    )
    LAST_RESULTS = res

    blocks = [res.results[c]["out"] for c in range(meta["ncores"])]
    full = np.concatenate(blocks, axis=0)
    return np.ascontiguousarray(full[: meta["n"], : meta["cout"]]).astype(np.float32)



# revision 10
# speedup vs baseline: 1.1967x; 1.1967x over previous
"""Two-layer DGL-style GCN (norm='both') on 8 TRN2 NeuronCores.

Sharding: dst-node blocks of 12544 per core (98 tiles of 128 nodes).
Per layer: project (PE matmul) -> gather per-edge src rows (indirect DMA)
-> segment-sum via one-hot matmul into PSUM -> scale by in-degree isqrt.
Layer-2 projections are exchanged with an ncfw AllGather between layers.

kernel(**inputs) takes the full unsharded inputs and returns the full
output; all sharding happens inside.
"""

import math

import numpy as np

import concourse.bacc as bacc
import concourse.bass as bass
import concourse.bass_utils as bass_utils
import concourse.mybir as mybir
import concourse.tile as tile

P = 128

# Full-problem constants (the grading harness calls kernel() with these shapes).
N_NODES = 100000
N_EDGES = 1600000
C_IN = 128
C_HID = 128
C_OUT = 40
C_OUT_PAD = 64
N_CORES = 8

F16 = mybir.dt.float16
F32 = mybir.dt.float32
I32 = mybir.dt.int32

# set by test.py to request a profiled run
TRACE = False
LAST_RESULTS = None


# ---------------------------------------------------------------- host prep


def prep_inputs(x, edge_index, W1, W2, ncores):
    """Shard the full inputs -> (in_maps, meta)."""
    n, cin = x.shape
    chid = W1.shape[1]
    cout = W2.shape[1]
    coutp = max(P // 2, int(2 ** math.ceil(math.log2(max(cout, 1)))))
    e = edge_index.shape[1]

    ntiles_pc = math.ceil(n / (ncores * P))  # tiles per core
    nb = ntiles_pc * P  # nodes per core
    npad = nb * ncores
    ntt = ntiles_pc * ncores  # total dst tiles

    src = np.asarray(edge_index[0], dtype=np.int64)
    dst = np.asarray(edge_index[1], dtype=np.int64)

    deg_out = np.bincount(src, minlength=npad).astype(np.float32)
    deg_in = np.bincount(dst, minlength=npad).astype(np.float32)
    oi = 1.0 / np.sqrt(np.maximum(deg_out, 1.0))
    ii = 1.0 / np.sqrt(np.maximum(deg_in, 1.0))

    # fold the out-degree scale into x; send transposed fp16
    xs = np.asarray(x, dtype=np.float32) * oi[:n, None]
    xsT = np.zeros((cin, npad), dtype=np.float16)
    xsT[:, :n] = xs.T

    # bucket edges by dst tile, sort by src within a tile for HBM locality
    gt = dst // P
    order = np.lexsort((src, gt))
    gts = gt[order]
    srcs = src[order].astype(np.int32)
    dls = (dst[order] % P).astype(np.float32)

    counts = np.bincount(gts, minlength=ntt)
    n_c = max(1, int(math.ceil(counts.max() / P)))
    L = n_c * P

    starts = np.zeros(ntt + 1, dtype=np.int64)
    starts[1:] = np.cumsum(counts)
    pos = np.arange(e, dtype=np.int64) - starts[gts]
    flat = gts * L + pos

    src_arr = np.zeros(ntt * L, dtype=np.int32)  # pad: gather row 0 (harmless)
    dl_arr = np.full(ntt * L, -1.0, dtype=np.float32)  # pad: matches no slot
    src_arr[flat] = srcs
    dl_arr[flat] = dls

    # [ntt, L] -> [ncores, P, ntiles_pc * n_c] with [p, t*n_c + j] = edge j*128+p
    src_pc = (
        src_arr.reshape(ncores, ntiles_pc, n_c, P)
        .transpose(0, 3, 1, 2)
        .reshape(ncores, P, ntiles_pc * n_c)
        .copy()
    )
    dl_pc = (
        dl_arr.reshape(ncores, ntiles_pc, n_c, P)
        .transpose(0, 3, 1, 2)
        .reshape(ncores, P, ntiles_pc * n_c)
        .copy()
    )

    # per-partition scale vectors: [p, t] = value for node t*128+p of the block
    iio = (ii * oi).reshape(ncores, ntiles_pc, P).transpose(0, 2, 1).copy()
    ii2 = ii.reshape(ncores, ntiles_pc, P).transpose(0, 2, 1).copy()

    W1_16 = np.asarray(W1, dtype=np.float16)
    W2p = np.zeros((chid, coutp), dtype=np.float16)
    W2p[:, :cout] = np.asarray(W2, dtype=np.float16)

    iota = np.tile(np.arange(P, dtype=np.float32), (P, 1))
    ident = np.eye(P, dtype=np.float32)

    in_maps = [
        {
            "xsT": xsT,
            "src_idx": src_pc[c],
            "dstloc": dl_pc[c],
            "iio": iio[c].astype(np.float32),
            "ii2": ii2[c].astype(np.float32),
            "W1": W1_16,
            "W2": W2p,
            "iota": iota,
            "ident": ident,
        }
        for c in range(ncores)
    ]

    meta = dict(
        n=n, cin=cin, chid=chid, cout=cout, coutp=coutp,
        ncores=ncores, ntiles_pc=ntiles_pc, nb=nb, npad=npad, n_c=n_c,
    )
    return in_maps, meta


# ---------------------------------------------------------------- device program


def build_nc(meta, debug=False, enable_asserts=False):
    cin = meta["cin"]
    chid = meta["chid"]
    coutp = meta["coutp"]
    ncores = meta["ncores"]
    ntiles_pc = meta["ntiles_pc"]
    nb = meta["nb"]
    npad = meta["npad"]
    n_c = meta["n_c"]
    ntt = npad // P

    nc = bacc.Bacc(
        "TRN2",
        target_bir_lowering=False,
        debug=debug,
        enable_asserts=enable_asserts,
        num_devices=ncores,
    )

    xsT = nc.dram_tensor("xsT", [cin, npad], F16, kind="ExternalInput")
    src_idx = nc.dram_tensor("src_idx", [P, ntiles_pc * n_c], I32, kind="ExternalInput")
    dstloc = nc.dram_tensor("dstloc", [P, ntiles_pc * n_c], F32, kind="ExternalInput")
    iio = nc.dram_tensor("iio", [P, ntiles_pc], F32, kind="ExternalInput")
    ii2 = nc.dram_tensor("ii2", [P, ntiles_pc], F32, kind="ExternalInput")
    W1 = nc.dram_tensor("W1", [cin, chid], F16, kind="ExternalInput")
    W2 = nc.dram_tensor("W2", [chid, coutp], F16, kind="ExternalInput")
    iota_d = nc.dram_tensor("iota", [P, P], F32, kind="ExternalInput")
    ident_d = nc.dram_tensor("ident", [P, P], F32, kind="ExternalInput")

    out = nc.dram_tensor("out", [nb, coutp], F32, kind="ExternalOutput")

    H1 = nc.dram_tensor("H1", [npad, chid], F16)
    H2b = nc.dram_tensor("H2b", [nb, coutp], F16)
    H2f = nc.dram_tensor("H2f", [npad, coutp], F16, addr_space="Shared")

    GRP = 4
    assert ntt % GRP == 0

    with tile.TileContext(nc) as tc:
        with (
            tc.tile_pool(name="const", bufs=1) as cpool,
            tc.tile_pool(name="a_in", bufs=3) as apool,
            tc.tile_pool(name="a_out", bufs=3) as aopool,
            tc.tile_pool(name="a_ps", bufs=2, space="PSUM") as apspool,
            tc.tile_pool(name="gbuf", bufs=2) as gpool,
            tc.tile_pool(name="mbuf", bufs=4) as mpool,
            tc.tile_pool(name="agg_ps", bufs=2, space="PSUM") as pspool,
            tc.tile_pool(name="aux_ps", bufs=1, space="PSUM") as xpspool,
            tc.tile_pool(name="flush", bufs=3) as flpool,
        ):
            w1_sb = cpool.tile([cin, chid], F16)
            nc.sync.dma_start(w1_sb[:], W1.ap())
            w2_sb = cpool.tile([chid, coutp], F16)
            nc.sync.dma_start(w2_sb[:], W2.ap())
            iota_f = cpool.tile([P, P], F32)
            nc.sync.dma_start(iota_f[:], iota_d.ap())
            ident_f = cpool.tile([P, P], F32)
            nc.sync.dma_start(ident_f[:], ident_d.ap())
            idx_all = cpool.tile([P, ntiles_pc * n_c], I32)
            nc.sync.dma_start(idx_all[:], src_idx.ap())
            dl_all = cpool.tile([P, ntiles_pc * n_c], F32)
            nc.sync.dma_start(dl_all[:], dstloc.ap())
            iio_sb = cpool.tile([P, ntiles_pc], F32)
            nc.sync.dma_start(iio_sb[:], iio.ap())
            ii2_sb = cpool.tile([P, ntiles_pc], F32)
            nc.sync.dma_start(ii2_sb[:], ii2.ap())

            # ---- Phase A: H1 = xs @ W1 for all npad rows (replicated work)
            xsTv = xsT.ap().rearrange("c (g n) -> g c n", n=GRP * P)
            h1v = H1.ap().rearrange("(g i p) c -> g p i c", p=P, i=GRP)
            for g in range(ntt // GRP):
                xt = apool.tile([cin, GRP * P], F16)
                nc.sync.dma_start(xt[:], xsTv[g])
                hp = apspool.tile([P, GRP * chid], F32)
                for i in range(GRP):
                    nc.tensor.matmul(
                        hp[:, i * chid : (i + 1) * chid],
                        lhsT=xt[:, i * P : (i + 1) * P],
                        rhs=w1_sb[:],
                        start=True,
                        stop=True,
                    )
                hs = aopool.tile([P, GRP * chid], F16)
                nc.vector.tensor_copy(hs[:], hp[:])
                nc.sync.dma_start(
                    h1v[g], hs[:].rearrange("p (i c) -> p i c", i=GRP)
                )

            # ---- Phase B: layer-1 aggregate per dst tile + fused layer-2 project
            h1_ap = H1.ap()
            for t in range(ntiles_pc):
                G = gpool.tile([P, n_c * chid], F16, tag="g1")
                # HW indirect DMA honors ONE offset per dest partition, so
                # gather chunk-by-chunk (128 rows per instruction).
                for c in range(n_c):
                    nc.gpsimd.indirect_dma_start(
                        out=G[:, c * chid : (c + 1) * chid],
                        out_offset=None,
                        in_=h1_ap,
                        in_offset=bass.IndirectOffsetOnAxis(
                            ap=idx_all[:, t * n_c + c : t * n_c + c + 1], axis=0
                        ),
                    )
                ps = pspool.tile([P, chid], F32, tag="agg1")
                for c in range(n_c):
                    M = mpool.tile([P, P], F16)
                    nc.vector.tensor_scalar(
                        out=M[:],
                        in0=iota_f[:],
                        scalar1=dl_all[:, t * n_c + c : t * n_c + c + 1],
                        scalar2=None,
                        op0=mybir.AluOpType.is_equal,
                    )
                    nc.tensor.matmul(
                        ps[:],
                        lhsT=M[:],
                        rhs=G[:, c * chid : (c + 1) * chid],
                        start=(c == 0),
                        stop=(c == n_c - 1),
                    )
                # x2s = relu(agg * ii) * oi  (positive scales commute with relu)
                x2s = flpool.tile([P, chid], F32, tag="x2s")
                nc.scalar.activation(
                    x2s[:],
                    ps[:],
                    mybir.ActivationFunctionType.Relu,
                    scale=iio_sb[:, t : t + 1],
                )
                xtp = xpspool.tile([P, P], F32, tag="xt_ps")
                nc.tensor.transpose(xtp[:], x2s[:], ident_f[:])
                xts = flpool.tile([P, P], F16, tag="xts")
                nc.vector.tensor_copy(xts[:], xtp[:])
                h2p = xpspool.tile([P, coutp], F32, tag="h2_ps")
                nc.tensor.matmul(h2p[:], lhsT=xts[:], rhs=w2_sb[:], start=True, stop=True)
                h2s = flpool.tile([P, coutp], F16, tag="h2s")
                nc.vector.tensor_copy(h2s[:], h2p[:])
                nc.sync.dma_start(H2b.ap()[t * P : (t + 1) * P, :], h2s[:])

            # ---- Phase C: exchange layer-2 projections
            nc.gpsimd.collective_compute(
                "AllGather",
                mybir.AluOpType.bypass,
                replica_groups=[list(range(ncores))],
                ins=[H2b.ap().opt()],
                outs=[H2f.ap().opt()],
            )

            # ---- Phase D: layer-2 aggregate per dst tile
            h2f_ap = H2f.ap()
            for t in range(ntiles_pc):
                G2 = gpool.tile([P, n_c * coutp], F16, tag="g2")
                for c in range(n_c):
                    nc.gpsimd.indirect_dma_start(
                        out=G2[:, c * coutp : (c + 1) * coutp],
                        out_offset=None,
                        in_=h2f_ap,
                        in_offset=bass.IndirectOffsetOnAxis(
                            ap=idx_all[:, t * n_c + c : t * n_c + c + 1], axis=0
                        ),
                    )
                ps2 = pspool.tile([P, coutp], F32, tag="agg2")
                for c in range(n_c):
                    M = mpool.tile([P, P], F16)
                    nc.vector.tensor_scalar(
                        out=M[:],
                        in0=iota_f[:],
                        scalar1=dl_all[:, t * n_c + c : t * n_c + c + 1],
                        scalar2=None,
                        op0=mybir.AluOpType.is_equal,
                    )
                    nc.tensor.matmul(
                        ps2[:],
                        lhsT=M[:],
                        rhs=G2[:, c * coutp : (c + 1) * coutp],
                        start=(c == 0),
                        stop=(c == n_c - 1),
                    )
                osb = flpool.tile([P, coutp], F32, tag="osb")
                nc.vector.tensor_scalar(
                    out=osb[:],
                    in0=ps2[:],
                    scalar1=ii2_sb[:, t : t + 1],
                    scalar2=None,
                    op0=mybir.AluOpType.mult,
                )
                nc.sync.dma_start(out.ap()[t * P : (t + 1) * P, :], osb[:])

    nc.compile()
    return nc


# ---------------------------------------------------------------- entry point

_CACHE = {}


def kernel(x, edge_index, W1, W2):
    global LAST_RESULTS
    x = np.asarray(x)
    edge_index = np.asarray(edge_index)
    W1 = np.asarray(W1)
    W2 = np.asarray(W2)

    in_maps, meta = prep_inputs(x, edge_index, W1, W2, N_CORES)

    key = (meta["npad"], meta["n_c"], meta["coutp"])
    nc = _CACHE.get(key)
    if nc is None:
        nc = build_nc(meta, debug=False, enable_asserts=False)
        _CACHE[key] = nc

    res = bass_utils.run_bass_kernel_spmd(
        nc,
        in_maps,
        core_ids=list(range(meta["ncores"])),
        trace=TRACE,
    )
    LAST_RESULTS = res

    blocks = [res.results[c]["out"] for c in range(meta["ncores"])]
    full = np.concatenate(blocks, axis=0)
    return np.ascontiguousarray(full[: meta["n"], : meta["cout"]]).astype(np.float32)

